# revision 32
# baseline (speedup 1.0000x reference)
"""Trainium2 Bass kernel for BinaryAssociativeMemory.

Sharding: phase 1 is head-parallel (8 cores x 2 heads): fused QKV
projections (float32r) + chunked linear-attention recurrence, producing the
pre-output-projection tensor in [head_dim, tokens] layout plus the final
state. Host reshards; phase 2 is token-parallel: out @ Wo.T in bf16.
"""

import numpy as np
import ml_dtypes

import bass_rust
import concourse.bass as bass
import concourse.mybir as mybir
import concourse.tile as tile
from concourse import bass_utils

# ---- problem constants (hardcoded per harness contract) ----
N_HEADS = 16
D_HEAD = 128
D_MODEL = 2048
CHUNK = 128
B = 4
T = 4096
SCALE = 1.0 / np.sqrt(D_HEAD)
N_CORES = 8
H_LOC = N_HEADS // N_CORES  # heads per core

F32 = mybir.dt.float32
F32R = mybir.dt.float32r
F16 = mybir.dt.float16
BF16 = mybir.dt.bfloat16
BF16_NP = ml_dtypes.bfloat16


# ---- walrus workaround: split multi-wait tail drain into 1-wait nops ----
def _patched_drain_and_barrier(self, tick_clock, wait_clock):
    from concourse.tile import ScopedClock

    nc = self.nc
    drain_inst = nc.sync.drain()
    wait_clock.add_sem_waits(
        drain_inst.ins, ScopedClock({None: tick_clock.global_clock})
    )
    si = drain_inst.ins.sync_info
    waits = list(si.on_wait) if si is not None else []
    if len(waits) > 1:
        drain_inst.ins.sync_info = bass_rust.SyncInfo(
            on_wait=[waits[0]], on_update=list(si.on_update)
        )
        for w in waits[1:]:
            nop = nc.sync.nop(hint="tail_wait_split", nofuse=True)
            nop.ins.sync_info = bass_rust.SyncInfo(on_wait=[w], on_update=[])

    nc.all_engine_barrier()
    assert self.sems is not None
    popped = nc._tile_sem_poison_stack.pop()
    assert popped is self._sem_poison
    nc.clear_and_free_semaphores(list(self.sems.allocated().values()))
    nc.all_engine_barrier()


def apply_tile_patch():
    tile.TileContext._drain_and_barrier = _patched_drain_and_barrier


def split_multi_waits(nc):
    """This walrus build allows only one sync-wait per instruction: hoist
    extra waits into single-wait NOPs on the same engine just before it."""
    uid = 0
    for f in nc.m.functions:
        for bb in f.blocks:
            newl = []
            changed = False
            for ins in bb.instructions:
                si = ins.sync_info
                if si is not None and len(si.on_wait) > 1:
                    waits = list(si.on_wait)
                    for w in waits[:-1]:
                        nop = mybir.InstNoOp(
                            name=f"{ins.name}_wsplit{uid}",
                            engine=ins.engine,
                            bass_nofuse=True,
                            sync_info=mybir.SyncInfo(on_wait=[w], on_update=[]),
                        )
                        uid += 1
                        newl.append(nop)
                    ins.sync_info = mybir.SyncInfo(
                        on_wait=[waits[-1]], on_update=list(si.on_update)
                    )
                    changed = True
                newl.append(ins)
            if changed:
                bb.instructions = newl
    return nc


# =====================================================================
# Phase 1 emitter: head-sharded fused QKV projection + chunked recurrence
# =====================================================================
def emit_k1(tc, io, D, NB, TOK_B, HL, GROUP, split3):
    nc = tc.nc
    KK = D // 128
    CG = GROUP // CHUNK
    G = TOK_B // GROUP
    NCHUNK_B = TOK_B // CHUNK
    outp, fstate = io["outp"], io["fstate"]

    def rv(name):
        return io[name].rearrange("(kk p) e -> p kk e", p=128)

    with (
        tc.tile_pool(name="const", bufs=1) as constp,
        tc.tile_pool(name="wts", bufs=1) as wtp,
        tc.tile_pool(name="xin", bufs=2) as xin,
        tc.tile_pool(name="proj", bufs=4) as proj,
        tc.tile_pool(name="recs", bufs=3) as recs,
        tc.tile_pool(name="stat", bufs=2) as statp,
        tc.tile_pool(name="oout", bufs=3) as oout,
        tc.tile_pool(name="qkps", bufs=2, space="PSUM") as qkps,
        tc.tile_pool(name="vps", bufs=2, space="PSUM") as vps,
        tc.tile_pool(name="recps", bufs=4, space="PSUM") as recps,
    ):
        id_sb = constp.tile([128, 128], BF16, name="id_sb")
        nc.sync.dma_start(id_sb[:], io["ident"][:])
        mask_sb = constp.tile([128, 128], F32, name="mask_sb")
        nc.sync.dma_start(mask_sb[:], io["mask"][:])

        if split3:
            wsb = {}
            for nm in ("wqh", "wql", "wkh", "wkl", "wvh", "wvl"):
                t = wtp.tile([128, KK, HL * 128], BF16, name=f"{nm}_sb")
                nc.sync.dma_start(t[:], rv(nm)[:])
                wsb[nm] = t
        else:
            wsb = {}
            for nm, wid, dt_ in (
                ("wq", HL * 128, BF16),
                ("wkv", 2 * HL * 128, F32R),
            ):
                t = wtp.tile([128, KK, wid], dt_, name=f"{nm}_sb")
                v = rv(nm)[:] if dt_ is BF16 else rv(nm)[:].bitcast(F32R)
                # split the load so early contraction chunks arrive first
                for qq in range(4):
                    kks = slice(qq * KK // 4, (qq + 1) * KK // 4)
                    nc.sync.dma_start(t[:, kks, :], v[:, kks, :])
                wsb[nm] = t

        for b in range(NB):
            # per-(b,h) recurrent state, fp32 accumulator + bf16 copy
            st32 = statp.tile([128, HL * 128], F32, name="st32")
            st16 = statp.tile([128, HL * 128], BF16, name="st16")

            for g in range(G):
                tok = b * TOK_B + g * GROUP
                if split3:
                    xh_v = io["xh"].rearrange("(kk p) t -> p kk t", p=128)
                    xl_v = io["xl"].rearrange("(kk p) t -> p kk t", p=128)
                    xhg = xin.tile([128, KK, GROUP], BF16, name="xhg", tag="xh")
                    nc.sync.dma_start(xhg[:], xh_v[:, :, tok : tok + GROUP])
                    xlg = xin.tile([128, KK, GROUP], BF16, name="xlg", tag="xl")
                    nc.sync.dma_start(xlg[:], xl_v[:, :, tok : tok + GROUP])

                    # term list for W-stationary projections: (w_key, x_tile)
                    def wx_terms(wn):
                        return [
                            (wsb[wn + "h"], xhg),
                            (wsb[wn + "l"], xhg),
                            (wsb[wn + "h"], xlg),
                        ]
                else:
                    xt_v = io["xt"].rearrange("(kk p) t -> p kk t", p=128)
                    xtg = xin.tile([128, KK, GROUP], F32R, name="xtg", tag="xtg")
                    xv = xt_v[:, :, tok : tok + GROUP].bitcast(F32R)
                    for qq in range(4):
                        kks = slice(qq * KK // 4, (qq + 1) * KK // 4)
                        nc.sync.dma_start(xtg[:, kks, :], xv[:, kks, :])
                    xb_v = io["xb"].rearrange("(kk p) t -> p kk t", p=128)
                    xbg = xin.tile([128, KK, GROUP], BF16, name="xbg", tag="xbg")
                    xbv = xb_v[:, :, tok : tok + GROUP]
                    for qq in range(4):
                        kks = slice(qq * KK // 4, (qq + 1) * KK // 4)
                        nc.sync.dma_start(xbg[:, kks, :], xbv[:, kks, :])

                    def wx_terms(wn):
                        return [(wsb[wn], xbg)]

                # --- projections: qT, kT ([d, t]) per head ---
                qt_sb = []
                kt_sb = []
                for h in range(HL):
                    hs = slice(h * 128, (h + 1) * 128)
                    qt_ps = qkps.tile([128, GROUP], F32, name="qt_ps", tag="qk")
                    terms = wx_terms("wq")
                    nt = len(terms)
                    for ti, (w, xg) in enumerate(terms):
                        for kk in range(KK):
                            nc.tensor.matmul(
                                qt_ps[:],
                                w[:, kk, hs],
                                xg[:, kk, :],
                                start=(ti == 0 and kk == 0),
                                stop=(ti == nt - 1 and kk == KK - 1),
                            )
                    qt = proj.tile([128, GROUP], BF16, name="qt", tag="qt")
                    nc.scalar.copy(qt[:], qt_ps[:])
                    qt_sb.append(qt)

                    if split3:
                        kt_ps = qkps.tile([128, GROUP], F32, name="kt_ps", tag="qk")
                        terms = wx_terms("wk")
                        for ti, (w, xg) in enumerate(terms):
                            for kk in range(KK):
                                nc.tensor.matmul(
                                    kt_ps[:],
                                    w[:, kk, hs],
                                    xg[:, kk, :],
                                    start=(ti == 0 and kk == 0),
                                    stop=(ti == nt - 1 and kk == KK - 1),
                                )
                        kt = proj.tile([128, GROUP], BF16, name="kt", tag="kt")
                        nc.scalar.sign(kt[:], kt_ps[:])
                        kt_sb.append(kt)

                o_sb = [
                    oout.tile([128, GROUP], BF16, name=f"o_sb{h}", tag=f"o{h}")
                    for h in range(HL)
                ]

                for c in range(CG):
                    ci = g * CG + c  # chunk index within batch b
                    cs = slice(c * 128, (c + 1) * 128)
                    if split3:
                        # --- v chunk, token-major [t, e] both heads ---
                        # x-stationary: terms are (x_chunk, w) pairs
                        vterms = [
                            (xhg, wsb["wvh"]),
                            (xhg, wsb["wvl"]),
                            (xlg, wsb["wvh"]),
                        ]
                        nvt = len(vterms)
                        v_ps = vps.tile([128, HL * 128], F32, name="v_ps")
                        for ti, (xg, w) in enumerate(vterms):
                            for kk in range(KK):
                                nc.tensor.matmul(
                                    v_ps[:],
                                    xg[:, kk, cs],
                                    w[:, kk, :],
                                    start=(ti == 0 and kk == 0),
                                    stop=(ti == nvt - 1 and kk == KK - 1),
                                )
                        v_sb = proj.tile([128, HL * 128], BF16, name="v_sb", tag="v")
                        nc.scalar.sign(v_sb[:], v_ps[:])
                    else:
                        # --- fused k|v chunk, token-major [t, 2*HL*128] ---
                        kv_ps = vps.tile([128, 2 * HL * 128], F32, name="kvp")
                        for kk in range(KK):
                            nc.tensor.matmul(
                                kv_ps[:],
                                xtg[:, kk, cs],
                                wsb["wkv"][:, kk, :],
                                start=(kk == 0),
                                stop=(kk == KK - 1),
                            )
                        kv_sb = proj.tile(
                            [128, 2 * HL * 128], BF16, name="kv_sb", tag="kv"
                        )
                        nc.scalar.sign(kv_sb[:], kv_ps[:])
                        v_sb = kv_sb[:, HL * 128 :]

                    for h in range(HL):
                        hs = slice(h * 128, (h + 1) * 128)
                        if split3:
                            # kc = (kT chunk)^T via PE transpose -> [t, d]
                            kc_ps = recps.tile(
                                [128, 128], BF16, name="kc_ps", tag="rec"
                            )
                            nc.tensor.transpose(
                                kc_ps[:], kt_sb[h][:, cs], id_sb[:]
                            )
                            kc = recs.tile([128, 128], BF16, name="kc", tag="kc")
                            nc.vector.tensor_copy(kc[:], kc_ps[:])
                            kt_ap = kt_sb[h][:, cs]
                            kc_ap = kc[:]
                        else:
                            # k arrives token-major; kT via PE transpose
                            kt_ps = recps.tile(
                                [128, 128], BF16, name="kt_ps", tag="rec"
                            )
                            nc.tensor.transpose(kt_ps[:], kv_sb[:, hs], id_sb[:])
                            kt = recs.tile([128, 128], BF16, name="kt", tag="kc")
                            nc.vector.tensor_copy(kt[:], kt_ps[:])
                            kt_ap = kt[:]
                            kc_ap = kv_sb[:, hs]

                        # scoresT[j,i] = sum_d kT[d,j] qT[d,i], masked j<=i
                        sc_ps = recps.tile([128, 128], F32, name="sc_ps", tag="rec")
                        nc.tensor.matmul(
                            sc_ps[:], kt_ap, qt_sb[h][:, cs],
                            start=True, stop=True,
                        )
                        scm = recs.tile([128, 128], BF16, name="scm", tag="scm")
                        nc.vector.tensor_mul(scm[:], sc_ps[:], mask_sb[:])

                        # out chunk [e, t]: crossT + intraT
                        o_ps = recps.tile([128, 128], F32, name="o_ps", tag="rec")
                        if ci > 0:
                            nc.tensor.matmul(
                                o_ps[:], st16[:, hs], qt_sb[h][:, cs],
                                start=True, stop=False,
                            )
                        nc.tensor.matmul(
                            o_ps[:], v_sb[:, hs], scm[:],
                            start=(ci == 0), stop=True,
                        )
                        nc.scalar.copy(o_sb[h][:, cs], o_ps[:])

                        # state update: kv[i,j] = sum_t kc[t,i] v[t,j]
                        kv_ps = recps.tile([128, 128], F32, name="kv_ps", tag="rec")
                        nc.tensor.matmul(
                            kv_ps[:], kc_ap, v_sb[:, hs], start=True, stop=True
                        )
                        if ci == 0:
                            nc.vector.tensor_copy(st32[:, hs], kv_ps[:])
                        else:
                            nc.vector.tensor_add(st32[:, hs], st32[:, hs], kv_ps[:])
                        if ci < NCHUNK_B - 1:
                            nc.scalar.copy(st16[:, hs], st32[:, hs])

                for h in range(HL):
                    nc.sync.dma_start(
                        outp[h * 128 : (h + 1) * 128, tok : tok + GROUP],
                        o_sb[h][:],
                    )

            for h in range(HL):
                nc.sync.dma_start(fstate[b, h], st32[:, h * 128 : (h + 1) * 128])


def build_k1(D=D_MODEL, NB=B, TOK_B=T, HL=H_LOC, GROUP=512, split3=True):
    """Per-core phase-1 program. See emit_k1 for IO contract."""
    apply_tile_patch()
    NTOK = NB * TOK_B
    nc = bass.Bass("TRN2", target_bir_lowering=False, debug=False)
    io = {}
    if split3:
        for nm in ("xh", "xl"):
            io[nm] = nc.dram_tensor(nm, [D, NTOK], BF16, kind="ExternalInput").ap()
        for nm in ("wqh", "wql", "wkh", "wkl", "wvh", "wvl"):
            io[nm] = nc.dram_tensor(
                nm, [D, HL * 128], BF16, kind="ExternalInput"
            ).ap()
    else:
        io["xt"] = nc.dram_tensor("xt", [D, NTOK], F32, kind="ExternalInput").ap()
        io["xb"] = nc.dram_tensor("xb", [D, NTOK], BF16, kind="ExternalInput").ap()
        io["wq"] = nc.dram_tensor(
            "wq", [D, HL * 128], BF16, kind="ExternalInput"
        ).ap()
        io["wkv"] = nc.dram_tensor(
            "wkv", [D, 2 * HL * 128], F32, kind="ExternalInput"
        ).ap()
    io["ident"] = nc.dram_tensor("ident", [128, 128], BF16, kind="ExternalInput").ap()
    io["mask"] = nc.dram_tensor("mask", [128, 128], F32, kind="ExternalInput").ap()
    io["outp"] = nc.dram_tensor(
        "outp", [HL * 128, NTOK], BF16, kind="ExternalOutput"
    ).ap()
    io["fstate"] = nc.dram_tensor(
        "fstate", [NB, HL, 128, 128], F32, kind="ExternalOutput"
    ).ap()
    with tile.TileContext(nc) as tc:
        emit_k1(tc, io, D, NB, TOK_B, HL, GROUP, split3)
    return split_multi_waits(nc)


# =====================================================================
# Phase 2 emitter: token-sharded output projection fin[t,e] = pre[:,t].T @ wo
# =====================================================================
def emit_k2(tc, pre, wo, fin, D, TLOC, EOUT):
    nc = tc.nc
    KK = D // 128
    TT = TLOC // 128
    EB = 512 if EOUT % 512 == 0 else 256
    ET = EOUT // EB

    pre_v = pre.rearrange("(kk p) t -> p kk t", p=128)
    wo_v = wo.rearrange("(kk p) e -> p kk e", p=128)

    with (
        tc.tile_pool(name="wts", bufs=1) as wtp,
        tc.tile_pool(name="pin", bufs=1) as pin,
        tc.tile_pool(name="fout", bufs=3) as fout,
        tc.tile_pool(name="ps", bufs=4, space="PSUM") as ps,
    ):
        wo_sb = wtp.tile([128, KK, EOUT], BF16, name="wo_sb")
        pre_sb = pin.tile([128, KK, TLOC], BF16, name="pre_sb")

        # loads ordered by first use: wo[ee0], pre[tt0..1], rest interleaved
        def load_wo(ee):
            es = slice(ee * EB, (ee + 1) * EB)
            for hh in range(4):
                ks = slice(hh * KK // 4, (hh + 1) * KK // 4)
                nc.sync.dma_start(wo_sb[:, ks, es], wo_v[:, ks, es])

        def load_pre(tt):
            ts_ = slice(tt * 128, (tt + 1) * 128)
            for hh in range(4):
                ks = slice(hh * KK // 4, (hh + 1) * KK // 4)
                nc.sync.dma_start(pre_sb[:, ks, ts_], pre_v[:, ks, ts_])

        load_wo(0)
        load_pre(0)
        load_pre(1)
        for ee in range(1, ET):
            load_wo(ee)
        for tt in range(2, TT):
            load_pre(tt)

        for tt in range(TT):
            ts_ = slice(tt * 128, (tt + 1) * 128)
            f_sb = fout.tile([128, EOUT], F32, name="f_sb")
            for ee in range(ET):
                es = slice(ee * EB, (ee + 1) * EB)
                f_ps = ps.tile([128, EB], F32, name="f_ps")
                for kk in range(KK):
                    nc.tensor.matmul(
                        f_ps[:],
                        pre_sb[:, kk, ts_],
                        wo_sb[:, kk, es],
                        start=(kk == 0),
                        stop=(kk == KK - 1),
                    )
                nc.scalar.copy(f_sb[:, es], f_ps[:])
            nc.sync.dma_start(fin[ts_, :], f_sb[:])


def build_k2(D=D_MODEL, TLOC=T * B // N_CORES, EOUT=D_MODEL):
    apply_tile_patch()
    nc = bass.Bass("TRN2", target_bir_lowering=False, debug=False)
    pre = nc.dram_tensor("pre", [D, TLOC], BF16, kind="ExternalInput").ap()
    wo = nc.dram_tensor("wo", [D, EOUT], BF16, kind="ExternalInput").ap()
    fin = nc.dram_tensor("fin", [TLOC, EOUT], F32, kind="ExternalOutput").ap()
    with tile.TileContext(nc) as tc:
        emit_k2(tc, pre, wo, fin, D, TLOC, EOUT)
    return split_multi_waits(nc)


# =====================================================================
# Host orchestration
# =====================================================================
_CACHE = {}


def _install_ntff_hook():
    """Provide antenv.axon_hooks (absent in this image) so trace=True can
    capture NTFF profiles through the axon tunnel."""
    import sys, types

    if "antenv.axon_hooks" in sys.modules:
        return
    try:
        from trn_agent_boot.trn_boot import _ntff_profile_via_ctypes

        hook = _ntff_profile_via_ctypes("/opt/axon/libaxon_pjrt.so")
    except Exception:
        hook = None
    mod = types.ModuleType("antenv.axon_hooks")
    mod.get_axon_ntff_profile_hook = lambda: hook
    mod.set_axon_ntff_profile_hook = lambda h: None
    sys.modules["antenv.axon_hooks"] = mod


SPLIT3 = False


def _get_programs():
    if "k1" not in _CACHE:
        _CACHE["k1"] = build_k1(split3=SPLIT3)
        _CACHE["k2"] = build_k2()
    return _CACHE["k1"], _CACHE["k2"]


def _bf16_split(a):
    """a (f32) -> (hi, lo) bf16 with hi + lo ~= a to ~16 mantissa bits."""
    hi = a.astype(BF16_NP)
    lo = (a - hi.astype(np.float32)).astype(BF16_NP)
    return hi, lo


def _run_with_retry(nc, in_maps, trace, attempts=3):
    import time as _time

    last = None
    for i in range(attempts):
        try:
            return bass_utils.run_bass_kernel_spmd(
                nc, in_maps, core_ids=list(range(N_CORES)), trace=trace
            )
        except Exception as e:  # transient NRT / device errors
            last = e
            _time.sleep(2.0 * (i + 1))
    raise last


def kernel(x, Wq, Wk, Wv, Wo, _trace=False):
    x = np.asarray(x, dtype=np.float32)
    Wq = np.asarray(Wq, dtype=np.float32)
    Wk = np.asarray(Wk, dtype=np.float32)
    Wv = np.asarray(Wv, dtype=np.float32)
    Wo = np.asarray(Wo, dtype=np.float32)

    if _trace:
        _install_ntff_hook()

    nc1, nc2 = _get_programs()

    xt = np.ascontiguousarray(x.reshape(B * T, D_MODEL).T)  # [D, NTOK]
    ident = np.eye(128, dtype=BF16_NP)
    mask_t = np.triu(np.ones((128, 128), dtype=np.float32))  # [j,i]=1 if j<=i

    in_maps1 = []
    if not SPLIT3:
        xb = xt.astype(BF16_NP)
    if SPLIT3:
        xh, xl = _bf16_split(xt)
        for c in range(N_CORES):
            hs = slice(c * H_LOC * 128, (c + 1) * H_LOC * 128)
            wqh, wql = _bf16_split(np.ascontiguousarray((Wq[hs, :] * SCALE).T))
            wkh, wkl = _bf16_split(np.ascontiguousarray(Wk[hs, :].T))
            wvh, wvl = _bf16_split(np.ascontiguousarray(Wv[hs, :].T))
            in_maps1.append(
                {
                    "xh": xh, "xl": xl,
                    "wqh": wqh, "wql": wql,
                    "wkh": wkh, "wkl": wkl,
                    "wvh": wvh, "wvl": wvl,
                    "ident": ident, "mask": mask_t,
                }
            )
    else:
        for c in range(N_CORES):
            hs = slice(c * H_LOC * 128, (c + 1) * H_LOC * 128)
            in_maps1.append(
                {
                    "xt": xt,
                    "xb": xb,
                    "wq": np.ascontiguousarray((Wq[hs, :] * SCALE).T).astype(
                        BF16_NP
                    ),
                    "wkv": np.ascontiguousarray(
                        np.concatenate([Wk[hs, :].T, Wv[hs, :].T], axis=1)
                    ),
                    "ident": ident,
                    "mask": mask_t,
                }
            )
    res1 = _run_with_retry(nc1, in_maps1, _trace)
    t1 = res1.exec_time_ns

    # reshard: stack per-core [HL*128, NTOK] -> [D, NTOK], slice tokens
    pre_full = np.concatenate(
        [np.asarray(res1.results[c]["outp"]) for c in range(N_CORES)], axis=0
    )
    wo_t = np.ascontiguousarray(Wo.T).astype(BF16_NP)
    TLOC = B * T // N_CORES
    in_maps2 = []
    for c in range(N_CORES):
        in_maps2.append(
            {
                "pre": np.ascontiguousarray(pre_full[:, c * TLOC : (c + 1) * TLOC]),
                "wo": wo_t,
            }
        )
    res2 = _run_with_retry(nc2, in_maps2, _trace)
    t2 = res2.exec_time_ns

    out = np.concatenate(
        [np.asarray(res2.results[c]["fin"]) for c in range(N_CORES)], axis=0
    ).reshape(B, T, D_MODEL)

    fst = np.empty((B, N_HEADS, D_HEAD, D_HEAD), dtype=np.float32)
    for c in range(N_CORES):
        fst[:, c * H_LOC : (c + 1) * H_LOC] = res1.results[c]["fstate"]

    if _trace:
        kernel.last_exec_ns = ((t1 or 0), (t2 or 0))
    return out, fst


# revision 33
# speedup vs baseline: 1.0057x; 1.0057x over previous
"""Trainium2 Bass kernel for BinaryAssociativeMemory.

Sharding: phase 1 is head-parallel (8 cores x 2 heads): q projection in
bf16, fused k|v projection in float32r (fp22 -- sign fidelity for the
binarization), plus the chunked linear-attention recurrence, producing the
pre-output-projection tensor in [head_dim, tokens] layout and the final
state. Host reshards; phase 2 is token-parallel: out @ Wo.T in bf16.
"""

import numpy as np
import ml_dtypes

import bass_rust
import concourse.bass as bass
import concourse.mybir as mybir
import concourse.tile as tile
from concourse import bass_utils

# ---- problem constants (hardcoded per harness contract) ----
N_HEADS = 16
D_HEAD = 128
D_MODEL = 2048
CHUNK = 128
B = 4
T = 4096
SCALE = 1.0 / np.sqrt(D_HEAD)
N_CORES = 8
H_LOC = N_HEADS // N_CORES  # heads per core

F32 = mybir.dt.float32
F32R = mybir.dt.float32r
F16 = mybir.dt.float16
BF16 = mybir.dt.bfloat16
BF16_NP = ml_dtypes.bfloat16


# ---- walrus workaround: split multi-wait tail drain into 1-wait nops ----
def _patched_drain_and_barrier(self, tick_clock, wait_clock):
    from concourse.tile import ScopedClock

    nc = self.nc
    drain_inst = nc.sync.drain()
    wait_clock.add_sem_waits(
        drain_inst.ins, ScopedClock({None: tick_clock.global_clock})
    )
    si = drain_inst.ins.sync_info
    waits = list(si.on_wait) if si is not None else []
    if len(waits) > 1:
        drain_inst.ins.sync_info = bass_rust.SyncInfo(
            on_wait=[waits[0]], on_update=list(si.on_update)
        )
        for w in waits[1:]:
            nop = nc.sync.nop(hint="tail_wait_split", nofuse=True)
            nop.ins.sync_info = bass_rust.SyncInfo(on_wait=[w], on_update=[])

    nc.all_engine_barrier()
    assert self.sems is not None
    popped = nc._tile_sem_poison_stack.pop()
    assert popped is self._sem_poison
    nc.clear_and_free_semaphores(list(self.sems.allocated().values()))
    nc.all_engine_barrier()


def apply_tile_patch():
    tile.TileContext._drain_and_barrier = _patched_drain_and_barrier


def split_multi_waits(nc):
    """This walrus build allows only one sync-wait per instruction: hoist
    extra waits into single-wait NOPs on the same engine just before it."""
    uid = 0
    for f in nc.m.functions:
        for bb in f.blocks:
            newl = []
            changed = False
            for ins in bb.instructions:
                si = ins.sync_info
                if si is not None and len(si.on_wait) > 1:
                    waits = list(si.on_wait)
                    for w in waits[:-1]:
                        nop = mybir.InstNoOp(
                            name=f"{ins.name}_wsplit{uid}",
                            engine=ins.engine,
                            bass_nofuse=True,
                            sync_info=mybir.SyncInfo(on_wait=[w], on_update=[]),
                        )
                        uid += 1
                        newl.append(nop)
                    ins.sync_info = mybir.SyncInfo(
                        on_wait=[waits[-1]], on_update=list(si.on_update)
                    )
                    changed = True
                newl.append(ins)
            if changed:
                bb.instructions = newl
    return nc


# =====================================================================
# Phase 1 emitter: head-sharded fused QKV projection + chunked recurrence
# =====================================================================
def emit_k1(tc, io, D, NB, TOK_B, HL, GROUP, split3):
    nc = tc.nc
    KK = D // 128
    CG = GROUP // CHUNK
    G = TOK_B // GROUP
    NCHUNK_B = TOK_B // CHUNK
    outp, fstate = io["outp"], io["fstate"]

    def rv(name):
        return io[name].rearrange("(kk p) e -> p kk e", p=128)

    with (
        tc.tile_pool(name="const", bufs=1) as constp,
        tc.tile_pool(name="wts", bufs=1) as wtp,
        tc.tile_pool(name="xin", bufs=2) as xin,
        tc.tile_pool(name="proj", bufs=4) as proj,
        tc.tile_pool(name="recs", bufs=3) as recs,
        tc.tile_pool(name="stat", bufs=2) as statp,
        tc.tile_pool(name="oout", bufs=3) as oout,
        tc.tile_pool(name="qkps", bufs=2, space="PSUM") as qkps,
        tc.tile_pool(name="vps", bufs=2, space="PSUM") as vps,
        tc.tile_pool(name="recps", bufs=4, space="PSUM") as recps,
    ):
        id_sb = constp.tile([128, 128], BF16, name="id_sb")
        nc.sync.dma_start(id_sb[:], io["ident"][:])
        mask_sb = constp.tile([128, 128], F32, name="mask_sb")
        nc.sync.dma_start(mask_sb[:], io["mask"][:])

        if split3:
            wsb = {}
            for nm in ("wqh", "wql", "wkh", "wkl", "wvh", "wvl"):
                t = wtp.tile([128, KK, HL * 128], BF16, name=f"{nm}_sb")
                nc.sync.dma_start(t[:], rv(nm)[:])
                wsb[nm] = t
        else:
            wsb = {}
            for nm, wid, dt_ in (
                ("wq", HL * 128, BF16),
                ("wkv", 2 * HL * 128, F32R),
            ):
                t = wtp.tile([128, KK, wid], dt_, name=f"{nm}_sb")
                v = rv(nm)[:] if dt_ is BF16 else rv(nm)[:].bitcast(F32R)
                # split the load so early contraction chunks arrive first
                for qq in range(4):
                    kks = slice(qq * KK // 4, (qq + 1) * KK // 4)
                    nc.sync.dma_start(t[:, kks, :], v[:, kks, :])
                wsb[nm] = t

        for b in range(NB):
            # per-(b,h) recurrent state, fp32 accumulator + bf16 copy
            st32 = statp.tile([128, HL * 128], F32, name="st32")
            st16 = statp.tile([128, HL * 128], BF16, name="st16")

            for g in range(G):
                tok = b * TOK_B + g * GROUP
                if split3:
                    xh_v = io["xh"].rearrange("(kk p) t -> p kk t", p=128)
                    xl_v = io["xl"].rearrange("(kk p) t -> p kk t", p=128)
                    xhg = xin.tile([128, KK, GROUP], BF16, name="xhg", tag="xh")
                    nc.sync.dma_start(xhg[:], xh_v[:, :, tok : tok + GROUP])
                    xlg = xin.tile([128, KK, GROUP], BF16, name="xlg", tag="xl")
                    nc.sync.dma_start(xlg[:], xl_v[:, :, tok : tok + GROUP])

                    # term list for W-stationary projections: (w_key, x_tile)
                    def wx_terms(wn):
                        return [
                            (wsb[wn + "h"], xhg),
                            (wsb[wn + "l"], xhg),
                            (wsb[wn + "h"], xlg),
                        ]
                else:
                    xt_v = io["xt"].rearrange("(kk p) t -> p kk t", p=128)
                    xtg = xin.tile([128, KK, GROUP], F32R, name="xtg", tag="xtg")
                    xv = xt_v[:, :, tok : tok + GROUP].bitcast(F32R)
                    for qq in range(4):
                        kks = slice(qq * KK // 4, (qq + 1) * KK // 4)
                        nc.sync.dma_start(xtg[:, kks, :], xv[:, kks, :])
                    xb_v = io["xb"].rearrange("(kk p) t -> p kk t", p=128)
                    xbg = xin.tile([128, KK, GROUP], BF16, name="xbg", tag="xbg")
                    xbv = xb_v[:, :, tok : tok + GROUP]
                    for qq in range(4):
                        kks = slice(qq * KK // 4, (qq + 1) * KK // 4)
                        nc.sync.dma_start(xbg[:, kks, :], xbv[:, kks, :])

                    def wx_terms(wn):
                        return [(wsb[wn], xbg)]

                # --- projections: qT, kT ([d, t]) per head ---
                qt_sb = []
                kt_sb = []
                for h in range(HL):
                    hs = slice(h * 128, (h + 1) * 128)
                    qt_ps = qkps.tile([128, GROUP], F32, name="qt_ps", tag="qk")
                    terms = wx_terms("wq")
                    nt = len(terms)
                    for ti, (w, xg) in enumerate(terms):
                        for kk in range(KK):
                            nc.tensor.matmul(
                                qt_ps[:],
                                w[:, kk, hs],
                                xg[:, kk, :],
                                start=(ti == 0 and kk == 0),
                                stop=(ti == nt - 1 and kk == KK - 1),
                            )
                    qt = proj.tile([128, GROUP], BF16, name="qt", tag="qt")
                    nc.scalar.copy(qt[:], qt_ps[:])
                    qt_sb.append(qt)

                    if split3:
                        kt_ps = qkps.tile([128, GROUP], F32, name="kt_ps", tag="qk")
                        terms = wx_terms("wk")
                        for ti, (w, xg) in enumerate(terms):
                            for kk in range(KK):
                                nc.tensor.matmul(
                                    kt_ps[:],
                                    w[:, kk, hs],
                                    xg[:, kk, :],
                                    start=(ti == 0 and kk == 0),
                                    stop=(ti == nt - 1 and kk == KK - 1),
                                )
                        kt = proj.tile([128, GROUP], BF16, name="kt", tag="kt")
                        nc.scalar.sign(kt[:], kt_ps[:])
                        kt_sb.append(kt)

                o_sb = [
                    oout.tile([128, GROUP], BF16, name=f"o_sb{h}", tag=f"o{h}")
                    for h in range(HL)
                ]

                for c in range(CG):
                    ci = g * CG + c  # chunk index within batch b
                    cs = slice(c * 128, (c + 1) * 128)
                    if split3:
                        # --- v chunk, token-major [t, e] both heads ---
                        # x-stationary: terms are (x_chunk, w) pairs
                        vterms = [
                            (xhg, wsb["wvh"]),
                            (xhg, wsb["wvl"]),
                            (xlg, wsb["wvh"]),
                        ]
                        nvt = len(vterms)
                        v_ps = vps.tile([128, HL * 128], F32, name="v_ps")
                        for ti, (xg, w) in enumerate(vterms):
                            for kk in range(KK):
                                nc.tensor.matmul(
                                    v_ps[:],
                                    xg[:, kk, cs],
                                    w[:, kk, :],
                                    start=(ti == 0 and kk == 0),
                                    stop=(ti == nvt - 1 and kk == KK - 1),
                                )
                        v_sb = proj.tile([128, HL * 128], BF16, name="v_sb", tag="v")
                        nc.scalar.sign(v_sb[:], v_ps[:])
                    else:
                        # --- fused k|v chunk, token-major [t, 2*HL*128] ---
                        kv_ps = vps.tile([128, 2 * HL * 128], F32, name="kvp")
                        for kk in range(KK):
                            nc.tensor.matmul(
                                kv_ps[:],
                                xtg[:, kk, cs],
                                wsb["wkv"][:, kk, :],
                                start=(kk == 0),
                                stop=(kk == KK - 1),
                            )
                        kv_sb = proj.tile(
                            [128, 2 * HL * 128], BF16, name="kv_sb", tag="kv"
                        )
                        nc.scalar.sign(kv_sb[:], kv_ps[:])
                        v_sb = kv_sb[:, HL * 128 :]

                    for h in range(HL):
                        hs = slice(h * 128, (h + 1) * 128)
                        if split3:
                            # kc = (kT chunk)^T via PE transpose -> [t, d]
                            kc_ps = recps.tile(
                                [128, 128], BF16, name="kc_ps", tag="rec"
                            )
                            nc.tensor.transpose(
                                kc_ps[:], kt_sb[h][:, cs], id_sb[:]
                            )
                            kc = recs.tile([128, 128], BF16, name="kc", tag="kc")
                            nc.vector.tensor_copy(kc[:], kc_ps[:])
                            kt_ap = kt_sb[h][:, cs]
                            kc_ap = kc[:]
                        else:
                            # k arrives token-major; kT via PE transpose
                            kt_ps = recps.tile(
                                [128, 128], BF16, name="kt_ps", tag="rec"
                            )
                            nc.tensor.transpose(kt_ps[:], kv_sb[:, hs], id_sb[:])
                            kt = recs.tile([128, 128], BF16, name="kt", tag="kc")
                            nc.vector.tensor_copy(kt[:], kt_ps[:])
                            kt_ap = kt[:]
                            kc_ap = kv_sb[:, hs]

                        # scoresT[j,i] = sum_d kT[d,j] qT[d,i], masked j<=i
                        sc_ps = recps.tile([128, 128], F32, name="sc_ps", tag="rec")
                        nc.tensor.matmul(
                            sc_ps[:], kt_ap, qt_sb[h][:, cs],
                            start=True, stop=True,
                        )
                        scm = recs.tile([128, 128], BF16, name="scm", tag="scm")
                        nc.vector.tensor_mul(scm[:], sc_ps[:], mask_sb[:])

                        # out chunk [e, t]: crossT + intraT
                        o_ps = recps.tile([128, 128], F32, name="o_ps", tag="rec")
                        if ci > 0:
                            nc.tensor.matmul(
                                o_ps[:], st16[:, hs], qt_sb[h][:, cs],
                                start=True, stop=False,
                            )
                        nc.tensor.matmul(
                            o_ps[:], v_sb[:, hs], scm[:],
                            start=(ci == 0), stop=True,
                        )
                        nc.scalar.copy(o_sb[h][:, cs], o_ps[:])

                        # state update: kv[i,j] = sum_t kc[t,i] v[t,j]
                        kv_ps = recps.tile([128, 128], F32, name="kv_ps", tag="rec")
                        nc.tensor.matmul(
                            kv_ps[:], kc_ap, v_sb[:, hs], start=True, stop=True
                        )
                        if ci == 0:
                            nc.vector.tensor_copy(st32[:, hs], kv_ps[:])
                        else:
                            nc.vector.tensor_add(st32[:, hs], st32[:, hs], kv_ps[:])
                        if ci < NCHUNK_B - 1:
                            nc.scalar.copy(st16[:, hs], st32[:, hs])

                for h in range(HL):
                    nc.sync.dma_start(
                        outp[h * 128 : (h + 1) * 128, tok : tok + GROUP],
                        o_sb[h][:],
                    )

            for h in range(HL):
                nc.sync.dma_start(fstate[b, h], st32[:, h * 128 : (h + 1) * 128])


def build_k1(D=D_MODEL, NB=B, TOK_B=T, HL=H_LOC, GROUP=512, split3=True):
    """Per-core phase-1 program. See emit_k1 for IO contract."""
    apply_tile_patch()
    NTOK = NB * TOK_B
    nc = bass.Bass("TRN2", target_bir_lowering=False, debug=False)
    io = {}
    if split3:
        for nm in ("xh", "xl"):
            io[nm] = nc.dram_tensor(nm, [D, NTOK], BF16, kind="ExternalInput").ap()
        for nm in ("wqh", "wql", "wkh", "wkl", "wvh", "wvl"):
            io[nm] = nc.dram_tensor(
                nm, [D, HL * 128], BF16, kind="ExternalInput"
            ).ap()
    else:
        io["xt"] = nc.dram_tensor("xt", [D, NTOK], F32, kind="ExternalInput").ap()
        io["xb"] = nc.dram_tensor("xb", [D, NTOK], BF16, kind="ExternalInput").ap()
        io["wq"] = nc.dram_tensor(
            "wq", [D, HL * 128], BF16, kind="ExternalInput"
        ).ap()
        io["wkv"] = nc.dram_tensor(
            "wkv", [D, 2 * HL * 128], F32, kind="ExternalInput"
        ).ap()
    io["ident"] = nc.dram_tensor("ident", [128, 128], BF16, kind="ExternalInput").ap()
    io["mask"] = nc.dram_tensor("mask", [128, 128], F32, kind="ExternalInput").ap()
    io["outp"] = nc.dram_tensor(
        "outp", [HL * 128, NTOK], BF16, kind="ExternalOutput"
    ).ap()
    io["fstate"] = nc.dram_tensor(
        "fstate", [NB, HL, 128, 128], F32, kind="ExternalOutput"
    ).ap()
    with tile.TileContext(nc) as tc:
        emit_k1(tc, io, D, NB, TOK_B, HL, GROUP, split3)
    return split_multi_waits(nc)


# =====================================================================
# Phase 2 emitter: token-sharded output projection fin[t,e] = pre[:,t].T @ wo
# =====================================================================
def emit_k2(tc, pre, wo, fin, D, TLOC, EOUT):
    nc = tc.nc
    KK = D // 128
    TT = TLOC // 128
    EB = 512 if EOUT % 512 == 0 else 256
    ET = EOUT // EB

    pre_v = pre.rearrange("(kk p) t -> p kk t", p=128)
    wo_v = wo.rearrange("(kk p) e -> p kk e", p=128)

    with (
        tc.tile_pool(name="wts", bufs=1) as wtp,
        tc.tile_pool(name="pin", bufs=1) as pin,
        tc.tile_pool(name="fout", bufs=3) as fout,
        tc.tile_pool(name="ps", bufs=4, space="PSUM") as ps,
    ):
        wo_sb = wtp.tile([128, KK, EOUT], BF16, name="wo_sb")
        pre_sb = pin.tile([128, KK, TLOC], BF16, name="pre_sb")

        # loads ordered by first use: wo[ee0], pre[tt0..1], rest interleaved
        def load_wo(ee):
            es = slice(ee * EB, (ee + 1) * EB)
            for hh in range(2):
                ks = slice(hh * KK // 2, (hh + 1) * KK // 2)
                nc.sync.dma_start(wo_sb[:, ks, es], wo_v[:, ks, es])

        def load_pre(tt):
            ts_ = slice(tt * 128, (tt + 1) * 128)
            for hh in range(2):
                ks = slice(hh * KK // 2, (hh + 1) * KK // 2)
                nc.sync.dma_start(pre_sb[:, ks, ts_], pre_v[:, ks, ts_])

        load_wo(0)
        load_pre(0)
        load_pre(1)
        for ee in range(1, ET):
            load_wo(ee)
        for tt in range(2, TT):
            load_pre(tt)

        for tt in range(TT):
            ts_ = slice(tt * 128, (tt + 1) * 128)
            f_sb = fout.tile([128, EOUT], F32, name="f_sb")
            for ee in range(ET):
                es = slice(ee * EB, (ee + 1) * EB)
                f_ps = ps.tile([128, EB], F32, name="f_ps")
                for kk in range(KK):
                    nc.tensor.matmul(
                        f_ps[:],
                        pre_sb[:, kk, ts_],
                        wo_sb[:, kk, es],
                        start=(kk == 0),
                        stop=(kk == KK - 1),
                    )
                nc.scalar.copy(f_sb[:, es], f_ps[:])
            nc.sync.dma_start(fin[ts_, :], f_sb[:])


def build_k2(D=D_MODEL, TLOC=T * B // N_CORES, EOUT=D_MODEL):
    apply_tile_patch()
    nc = bass.Bass("TRN2", target_bir_lowering=False, debug=False)
    pre = nc.dram_tensor("pre", [D, TLOC], BF16, kind="ExternalInput").ap()
    wo = nc.dram_tensor("wo", [D, EOUT], BF16, kind="ExternalInput").ap()
    fin = nc.dram_tensor("fin", [TLOC, EOUT], F32, kind="ExternalOutput").ap()
    with tile.TileContext(nc) as tc:
        emit_k2(tc, pre, wo, fin, D, TLOC, EOUT)
    return split_multi_waits(nc)


# =====================================================================
# Host orchestration
# =====================================================================
_CACHE = {}


def _install_ntff_hook():
    """Provide antenv.axon_hooks (absent in this image) so trace=True can
    capture NTFF profiles through the axon tunnel."""
    import sys, types

    if "antenv.axon_hooks" in sys.modules:
        return
    try:
        from trn_agent_boot.trn_boot import _ntff_profile_via_ctypes

        hook = _ntff_profile_via_ctypes("/opt/axon/libaxon_pjrt.so")
    except Exception:
        hook = None
    mod = types.ModuleType("antenv.axon_hooks")
    mod.get_axon_ntff_profile_hook = lambda: hook
    mod.set_axon_ntff_profile_hook = lambda h: None
    sys.modules["antenv.axon_hooks"] = mod


SPLIT3 = False


def _get_programs():
    if "k1" not in _CACHE:
        _CACHE["k1"] = build_k1(split3=SPLIT3)
        _CACHE["k2"] = build_k2()
    return _CACHE["k1"], _CACHE["k2"]


def _bf16_split(a):
    """a (f32) -> (hi, lo) bf16 with hi + lo ~= a to ~16 mantissa bits."""
    hi = a.astype(BF16_NP)
    lo = (a - hi.astype(np.float32)).astype(BF16_NP)
    return hi, lo


def _run_with_retry(nc, in_maps, trace, attempts=3):
    import time as _time

    last = None
    for i in range(attempts):
        try:
            return bass_utils.run_bass_kernel_spmd(
                nc, in_maps, core_ids=list(range(N_CORES)), trace=trace
            )
        except Exception as e:  # transient NRT / device errors
            last = e
            _time.sleep(2.0 * (i + 1))
    raise last


def kernel(x, Wq, Wk, Wv, Wo, _trace=False):
    x = np.asarray(x, dtype=np.float32)
    Wq = np.asarray(Wq, dtype=np.float32)
    Wk = np.asarray(Wk, dtype=np.float32)
    Wv = np.asarray(Wv, dtype=np.float32)
    Wo = np.asarray(Wo, dtype=np.float32)

    if _trace:
        _install_ntff_hook()

    nc1, nc2 = _get_programs()

    xt = np.ascontiguousarray(x.reshape(B * T, D_MODEL).T)  # [D, NTOK]
    ident = np.eye(128, dtype=BF16_NP)
    mask_t = np.triu(np.ones((128, 128), dtype=np.float32))  # [j,i]=1 if j<=i

    in_maps1 = []
    if not SPLIT3:
        xb = xt.astype(BF16_NP)
    if SPLIT3:
        xh, xl = _bf16_split(xt)
        for c in range(N_CORES):
            hs = slice(c * H_LOC * 128, (c + 1) * H_LOC * 128)
            wqh, wql = _bf16_split(np.ascontiguousarray((Wq[hs, :] * SCALE).T))
            wkh, wkl = _bf16_split(np.ascontiguousarray(Wk[hs, :].T))
            wvh, wvl = _bf16_split(np.ascontiguousarray(Wv[hs, :].T))
            in_maps1.append(
                {
                    "xh": xh, "xl": xl,
                    "wqh": wqh, "wql": wql,
                    "wkh": wkh, "wkl": wkl,
                    "wvh": wvh, "wvl": wvl,
                    "ident": ident, "mask": mask_t,
                }
            )
    else:
        for c in range(N_CORES):
            hs = slice(c * H_LOC * 128, (c + 1) * H_LOC * 128)
            in_maps1.append(
                {
                    "xt": xt,
                    "xb": xb,
                    "wq": np.ascontiguousarray((Wq[hs, :] * SCALE).T).astype(
                        BF16_NP
                    ),
                    "wkv": np.ascontiguousarray(
                        np.concatenate([Wk[hs, :].T, Wv[hs, :].T], axis=1)
                    ),
                    "ident": ident,
                    "mask": mask_t,
                }
            )
    res1 = _run_with_retry(nc1, in_maps1, _trace)
    t1 = res1.exec_time_ns

    # reshard: stack per-core [HL*128, NTOK] -> [D, NTOK], slice tokens
    pre_full = np.concatenate(
        [np.asarray(res1.results[c]["outp"]) for c in range(N_CORES)], axis=0
    )
    wo_t = np.ascontiguousarray(Wo.T).astype(BF16_NP)
    TLOC = B * T // N_CORES
    in_maps2 = []
    for c in range(N_CORES):
        in_maps2.append(
            {
                "pre": np.ascontiguousarray(pre_full[:, c * TLOC : (c + 1) * TLOC]),
                "wo": wo_t,
            }
        )
    res2 = _run_with_retry(nc2, in_maps2, _trace)
    t2 = res2.exec_time_ns

    out = np.concatenate(
        [np.asarray(res2.results[c]["fin"]) for c in range(N_CORES)], axis=0
    ).reshape(B, T, D_MODEL)

    fst = np.empty((B, N_HEADS, D_HEAD, D_HEAD), dtype=np.float32)
    for c in range(N_CORES):
        fst[:, c * H_LOC : (c + 1) * H_LOC] = res1.results[c]["fstate"]

    if _trace:
        kernel.last_exec_ns = ((t1 or 0), (t2 or 0))
    return out, fst


# revision 35
# speedup vs baseline: 1.0128x; 1.0071x over previous
"""Trainium2 Bass kernel for BinaryAssociativeMemory.

Sharding: phase 1 is head-parallel (8 cores x 2 heads): q projection in
bf16, fused k|v projection in float32r (fp22 -- sign fidelity for the
binarization), plus the chunked linear-attention recurrence, producing the
pre-output-projection tensor in [head_dim, tokens] layout and the final
state. Host reshards; phase 2 is token-parallel: out @ Wo.T in bf16.
"""

import numpy as np
import ml_dtypes

import bass_rust
import concourse.bass as bass
import concourse.mybir as mybir
import concourse.tile as tile
from concourse import bass_utils

# ---- problem constants (hardcoded per harness contract) ----
N_HEADS = 16
D_HEAD = 128
D_MODEL = 2048
CHUNK = 128
B = 4
T = 4096
SCALE = 1.0 / np.sqrt(D_HEAD)
N_CORES = 8
H_LOC = N_HEADS // N_CORES  # heads per core

F32 = mybir.dt.float32
F32R = mybir.dt.float32r
F16 = mybir.dt.float16
BF16 = mybir.dt.bfloat16
BF16_NP = ml_dtypes.bfloat16


# ---- walrus workaround: split multi-wait tail drain into 1-wait nops ----
def _patched_drain_and_barrier(self, tick_clock, wait_clock):
    from concourse.tile import ScopedClock

    nc = self.nc
    drain_inst = nc.sync.drain()
    wait_clock.add_sem_waits(
        drain_inst.ins, ScopedClock({None: tick_clock.global_clock})
    )
    si = drain_inst.ins.sync_info
    waits = list(si.on_wait) if si is not None else []
    if len(waits) > 1:
        drain_inst.ins.sync_info = bass_rust.SyncInfo(
            on_wait=[waits[0]], on_update=list(si.on_update)
        )
        for w in waits[1:]:
            nop = nc.sync.nop(hint="tail_wait_split", nofuse=True)
            nop.ins.sync_info = bass_rust.SyncInfo(on_wait=[w], on_update=[])

    nc.all_engine_barrier()
    assert self.sems is not None
    popped = nc._tile_sem_poison_stack.pop()
    assert popped is self._sem_poison
    nc.clear_and_free_semaphores(list(self.sems.allocated().values()))
    nc.all_engine_barrier()


def apply_tile_patch():
    tile.TileContext._drain_and_barrier = _patched_drain_and_barrier


def split_multi_waits(nc):
    """This walrus build allows only one sync-wait per instruction: hoist
    extra waits into single-wait NOPs on the same engine just before it."""
    uid = 0
    for f in nc.m.functions:
        for bb in f.blocks:
            newl = []
            changed = False
            for ins in bb.instructions:
                si = ins.sync_info
                if si is not None and len(si.on_wait) > 1:
                    waits = list(si.on_wait)
                    for w in waits[:-1]:
                        nop = mybir.InstNoOp(
                            name=f"{ins.name}_wsplit{uid}",
                            engine=ins.engine,
                            bass_nofuse=True,
                            sync_info=mybir.SyncInfo(on_wait=[w], on_update=[]),
                        )
                        uid += 1
                        newl.append(nop)
                    ins.sync_info = mybir.SyncInfo(
                        on_wait=[waits[-1]], on_update=list(si.on_update)
                    )
                    changed = True
                newl.append(ins)
            if changed:
                bb.instructions = newl
    return nc


# =====================================================================
# Phase 1 emitter: head-sharded fused QKV projection + chunked recurrence
# =====================================================================
def emit_k1(tc, io, D, NB, TOK_B, HL, GROUP, split3):
    nc = tc.nc
    KK = D // 128
    CG = GROUP // CHUNK
    G = TOK_B // GROUP
    NCHUNK_B = TOK_B // CHUNK
    outp, fstate = io["outp"], io["fstate"]

    def rv(name):
        return io[name].rearrange("(kk p) e -> p kk e", p=128)

    with (
        tc.tile_pool(name="const", bufs=1) as constp,
        tc.tile_pool(name="wts", bufs=1) as wtp,
        tc.tile_pool(name="xin", bufs=2) as xin,
        tc.tile_pool(name="proj", bufs=4) as proj,
        tc.tile_pool(name="recs", bufs=3) as recs,
        tc.tile_pool(name="stat", bufs=2) as statp,
        tc.tile_pool(name="oout", bufs=3) as oout,
        tc.tile_pool(name="qkps", bufs=2, space="PSUM") as qkps,
        tc.tile_pool(name="vps", bufs=2, space="PSUM") as vps,
        tc.tile_pool(name="recps", bufs=4, space="PSUM") as recps,
    ):
        id_sb = constp.tile([128, 128], BF16, name="id_sb")
        nc.sync.dma_start(id_sb[:], io["ident"][:])
        mask_sb = constp.tile([128, 128], F32, name="mask_sb")
        nc.sync.dma_start(mask_sb[:], io["mask"][:])

        if split3:
            wsb = {}
            for nm in ("wqh", "wql", "wkh", "wkl", "wvh", "wvl"):
                t = wtp.tile([128, KK, HL * 128], BF16, name=f"{nm}_sb")
                nc.sync.dma_start(t[:], rv(nm)[:])
                wsb[nm] = t
        else:
            wsb = {}
            for nm, wid, dt_ in (
                ("wq", HL * 128, BF16),
                ("wkv", 2 * HL * 128, F32R),
            ):
                t = wtp.tile([128, KK, wid], dt_, name=f"{nm}_sb")
                v = rv(nm)[:] if dt_ is BF16 else rv(nm)[:].bitcast(F32R)
                # split the load so early contraction chunks arrive first
                for qq in range(4):
                    kks = slice(qq * KK // 4, (qq + 1) * KK // 4)
                    nc.sync.dma_start(t[:, kks, :], v[:, kks, :])
                wsb[nm] = t

        for b in range(NB):
            # per-(b,h) recurrent state, fp32 accumulator + bf16 copy
            st32 = statp.tile([128, HL * 128], F32, name="st32")
            st16 = statp.tile([128, HL * 128], BF16, name="st16")

            for g in range(G):
                tok = b * TOK_B + g * GROUP
                if split3:
                    xh_v = io["xh"].rearrange("(kk p) t -> p kk t", p=128)
                    xl_v = io["xl"].rearrange("(kk p) t -> p kk t", p=128)
                    xhg = xin.tile([128, KK, GROUP], BF16, name="xhg", tag="xh")
                    nc.sync.dma_start(xhg[:], xh_v[:, :, tok : tok + GROUP])
                    xlg = xin.tile([128, KK, GROUP], BF16, name="xlg", tag="xl")
                    nc.sync.dma_start(xlg[:], xl_v[:, :, tok : tok + GROUP])

                    # term list for W-stationary projections: (w_key, x_tile)
                    def wx_terms(wn):
                        return [
                            (wsb[wn + "h"], xhg),
                            (wsb[wn + "l"], xhg),
                            (wsb[wn + "h"], xlg),
                        ]
                else:
                    xt_v = io["xt"].rearrange("(kk p) t -> p kk t", p=128)
                    xtg = xin.tile([128, KK, GROUP], F32R, name="xtg", tag="xtg")
                    xv = xt_v[:, :, tok : tok + GROUP].bitcast(F32R)
                    for qq in range(4):
                        kks = slice(qq * KK // 4, (qq + 1) * KK // 4)
                        nc.sync.dma_start(xtg[:, kks, :], xv[:, kks, :])
                    xb_v = io["xb"].rearrange("(kk p) t -> p kk t", p=128)
                    xbg = xin.tile([128, KK, GROUP], BF16, name="xbg", tag="xbg")
                    xbv = xb_v[:, :, tok : tok + GROUP]
                    for qq in range(4):
                        kks = slice(qq * KK // 4, (qq + 1) * KK // 4)
                        nc.sync.dma_start(xbg[:, kks, :], xbv[:, kks, :])

                    def wx_terms(wn):
                        return [(wsb[wn], xbg)]

                # --- projections: qT, kT ([d, t]) per head ---
                qt_sb = []
                kt_sb = []
                for h in range(HL):
                    hs = slice(h * 128, (h + 1) * 128)
                    qt_ps = qkps.tile([128, GROUP], F32, name="qt_ps", tag="qk")
                    terms = wx_terms("wq")
                    nt = len(terms)
                    for ti, (w, xg) in enumerate(terms):
                        for kk in range(KK):
                            nc.tensor.matmul(
                                qt_ps[:],
                                w[:, kk, hs],
                                xg[:, kk, :],
                                start=(ti == 0 and kk == 0),
                                stop=(ti == nt - 1 and kk == KK - 1),
                            )
                    qt = proj.tile([128, GROUP], BF16, name="qt", tag="qt")
                    nc.scalar.copy(qt[:], qt_ps[:])
                    qt_sb.append(qt)

                    if split3:
                        kt_ps = qkps.tile([128, GROUP], F32, name="kt_ps", tag="qk")
                        terms = wx_terms("wk")
                        for ti, (w, xg) in enumerate(terms):
                            for kk in range(KK):
                                nc.tensor.matmul(
                                    kt_ps[:],
                                    w[:, kk, hs],
                                    xg[:, kk, :],
                                    start=(ti == 0 and kk == 0),
                                    stop=(ti == nt - 1 and kk == KK - 1),
                                )
                        kt = proj.tile([128, GROUP], BF16, name="kt", tag="kt")
                        nc.scalar.sign(kt[:], kt_ps[:])
                        kt_sb.append(kt)

                o_sb = [
                    oout.tile([128, GROUP], BF16, name=f"o_sb{h}", tag=f"o{h}")
                    for h in range(HL)
                ]

                for c in range(CG):
                    ci = g * CG + c  # chunk index within batch b
                    cs = slice(c * 128, (c + 1) * 128)
                    if split3:
                        # --- v chunk, token-major [t, e] both heads ---
                        # x-stationary: terms are (x_chunk, w) pairs
                        vterms = [
                            (xhg, wsb["wvh"]),
                            (xhg, wsb["wvl"]),
                            (xlg, wsb["wvh"]),
                        ]
                        nvt = len(vterms)
                        v_ps = vps.tile([128, HL * 128], F32, name="v_ps")
                        for ti, (xg, w) in enumerate(vterms):
                            for kk in range(KK):
                                nc.tensor.matmul(
                                    v_ps[:],
                                    xg[:, kk, cs],
                                    w[:, kk, :],
                                    start=(ti == 0 and kk == 0),
                                    stop=(ti == nvt - 1 and kk == KK - 1),
                                )
                        v_sb = proj.tile([128, HL * 128], BF16, name="v_sb", tag="v")
                        nc.scalar.sign(v_sb[:], v_ps[:])
                    else:
                        # --- fused k|v chunk, token-major [t, 2*HL*128] ---
                        kv_ps = vps.tile([128, 2 * HL * 128], F32, name="kvp")
                        for kk in range(KK):
                            nc.tensor.matmul(
                                kv_ps[:],
                                xtg[:, kk, cs],
                                wsb["wkv"][:, kk, :],
                                start=(kk == 0),
                                stop=(kk == KK - 1),
                            )
                        kv_sb = proj.tile(
                            [128, 2 * HL * 128], BF16, name="kv_sb", tag="kv"
                        )
                        nc.scalar.sign(kv_sb[:], kv_ps[:])
                        v_sb = kv_sb[:, HL * 128 :]

                    for h in range(HL):
                        hs = slice(h * 128, (h + 1) * 128)
                        if split3:
                            # kc = (kT chunk)^T via PE transpose -> [t, d]
                            kc_ps = recps.tile(
                                [128, 128], BF16, name="kc_ps", tag="rec"
                            )
                            nc.tensor.transpose(
                                kc_ps[:], kt_sb[h][:, cs], id_sb[:]
                            )
                            kc = recs.tile([128, 128], BF16, name="kc", tag="kc")
                            nc.vector.tensor_copy(kc[:], kc_ps[:])
                            kt_ap = kt_sb[h][:, cs]
                            kc_ap = kc[:]
                        else:
                            # k arrives token-major; kT via PE transpose
                            kt_ps = recps.tile(
                                [128, 128], BF16, name="kt_ps", tag="rec"
                            )
                            nc.tensor.transpose(kt_ps[:], kv_sb[:, hs], id_sb[:])
                            kt = recs.tile([128, 128], BF16, name="kt", tag="kc")
                            nc.vector.tensor_copy(kt[:], kt_ps[:])
                            kt_ap = kt[:]
                            kc_ap = kv_sb[:, hs]

                        # scoresT[j,i] = sum_d kT[d,j] qT[d,i], masked j<=i
                        sc_ps = recps.tile([128, 128], F32, name="sc_ps", tag="rec")
                        nc.tensor.matmul(
                            sc_ps[:], kt_ap, qt_sb[h][:, cs],
                            start=True, stop=True,
                        )
                        scm = recs.tile([128, 128], BF16, name="scm", tag="scm")
                        nc.vector.tensor_mul(scm[:], sc_ps[:], mask_sb[:])

                        # out chunk [e, t]: crossT + intraT
                        o_ps = recps.tile([128, 128], F32, name="o_ps", tag="rec")
                        if ci > 0:
                            nc.tensor.matmul(
                                o_ps[:], st16[:, hs], qt_sb[h][:, cs],
                                start=True, stop=False,
                            )
                        nc.tensor.matmul(
                            o_ps[:], v_sb[:, hs], scm[:],
                            start=(ci == 0), stop=True,
                        )
                        nc.scalar.copy(o_sb[h][:, cs], o_ps[:])

                        # state update: kv[i,j] = sum_t kc[t,i] v[t,j]
                        kv_ps = recps.tile([128, 128], F32, name="kv_ps", tag="rec")
                        nc.tensor.matmul(
                            kv_ps[:], kc_ap, v_sb[:, hs], start=True, stop=True
                        )
                        if ci == 0:
                            nc.vector.tensor_copy(st32[:, hs], kv_ps[:])
                        else:
                            nc.vector.tensor_add(st32[:, hs], st32[:, hs], kv_ps[:])
                        if ci < NCHUNK_B - 1:
                            nc.scalar.copy(st16[:, hs], st32[:, hs])

                for h in range(HL):
                    nc.sync.dma_start(
                        outp[h * 128 : (h + 1) * 128, tok : tok + GROUP],
                        o_sb[h][:],
                    )

            for h in range(HL):
                nc.sync.dma_start(fstate[b, h], st32[:, h * 128 : (h + 1) * 128])


def build_k1(D=D_MODEL, NB=B, TOK_B=T, HL=H_LOC, GROUP=512, split3=True):
    """Per-core phase-1 program. See emit_k1 for IO contract."""
    apply_tile_patch()
    NTOK = NB * TOK_B
    nc = bass.Bass("TRN2", target_bir_lowering=False, debug=False)
    io = {}
    if split3:
        for nm in ("xh", "xl"):
            io[nm] = nc.dram_tensor(nm, [D, NTOK], BF16, kind="ExternalInput").ap()
        for nm in ("wqh", "wql", "wkh", "wkl", "wvh", "wvl"):
            io[nm] = nc.dram_tensor(
                nm, [D, HL * 128], BF16, kind="ExternalInput"
            ).ap()
    else:
        io["xt"] = nc.dram_tensor("xt", [D, NTOK], F32, kind="ExternalInput").ap()
        io["xb"] = nc.dram_tensor("xb", [D, NTOK], BF16, kind="ExternalInput").ap()
        io["wq"] = nc.dram_tensor(
            "wq", [D, HL * 128], BF16, kind="ExternalInput"
        ).ap()
        io["wkv"] = nc.dram_tensor(
            "wkv", [D, 2 * HL * 128], F32, kind="ExternalInput"
        ).ap()
    io["ident"] = nc.dram_tensor("ident", [128, 128], BF16, kind="ExternalInput").ap()
    io["mask"] = nc.dram_tensor("mask", [128, 128], F32, kind="ExternalInput").ap()
    io["outp"] = nc.dram_tensor(
        "outp", [HL * 128, NTOK], BF16, kind="ExternalOutput"
    ).ap()
    io["fstate"] = nc.dram_tensor(
        "fstate", [NB, HL, 128, 128], F32, kind="ExternalOutput"
    ).ap()
    with tile.TileContext(nc) as tc:
        emit_k1(tc, io, D, NB, TOK_B, HL, GROUP, split3)
    return split_multi_waits(nc)


# =====================================================================
# Phase 2 emitter: token-sharded output projection fin[t,e] = pre[:,t].T @ wo
# =====================================================================
def emit_k2(tc, pre, wo, fin, D, TLOC, EOUT):
    nc = tc.nc
    KK = D // 128
    TT = TLOC // 128
    EB = 512 if EOUT % 512 == 0 else 256
    ET = EOUT // EB

    pre_v = pre.rearrange("(kk p) t -> p kk t", p=128)
    wo_v = wo.rearrange("(kk p) e -> p kk e", p=128)

    with (
        tc.tile_pool(name="wts", bufs=1) as wtp,
        tc.tile_pool(name="pin", bufs=1) as pin,
        tc.tile_pool(name="fout", bufs=3) as fout,
        tc.tile_pool(name="ps", bufs=4, space="PSUM") as ps,
    ):
        wo_sb = wtp.tile([128, KK, EOUT], BF16, name="wo_sb")
        pre_sb = pin.tile([128, KK, TLOC], BF16, name="pre_sb")

        # loads ordered by first use: wo[ee0], pre[tt0..1], rest interleaved
        def load_wo(ee):
            es = slice(ee * EB, (ee + 1) * EB)
            for hh in range(2):
                ks = slice(hh * KK // 2, (hh + 1) * KK // 2)
                nc.sync.dma_start(wo_sb[:, ks, es], wo_v[:, ks, es])

        def load_pre(tt):
            ts_ = slice(tt * 128, (tt + 1) * 128)
            for hh in range(2):
                ks = slice(hh * KK // 2, (hh + 1) * KK // 2)
                nc.sync.dma_start(pre_sb[:, ks, ts_], pre_v[:, ks, ts_])

        load_wo(0)
        load_pre(0)
        load_pre(1)
        for ee in range(1, ET):
            load_wo(ee)
        for tt in range(2, TT):
            load_pre(tt)

        for tt in range(TT):
            ts_ = slice(tt * 128, (tt + 1) * 128)
            f_sb = fout.tile([128, EOUT], F32, name="f_sb")
            for ee in range(ET):
                es = slice(ee * EB, (ee + 1) * EB)
                f_ps = ps.tile([128, EB], F32, name="f_ps")
                for kk in range(KK):
                    nc.tensor.matmul(
                        f_ps[:],
                        pre_sb[:, kk, ts_],
                        wo_sb[:, kk, es],
                        start=(kk == 0),
                        stop=(kk == KK - 1),
                    )
                nc.scalar.copy(f_sb[:, es], f_ps[:])
                nc.sync.dma_start(fin[ts_, es], f_sb[:, es])


def build_k2(D=D_MODEL, TLOC=T * B // N_CORES, EOUT=D_MODEL):
    apply_tile_patch()
    nc = bass.Bass("TRN2", target_bir_lowering=False, debug=False)
    pre = nc.dram_tensor("pre", [D, TLOC], BF16, kind="ExternalInput").ap()
    wo = nc.dram_tensor("wo", [D, EOUT], BF16, kind="ExternalInput").ap()
    fin = nc.dram_tensor("fin", [TLOC, EOUT], F32, kind="ExternalOutput").ap()
    with tile.TileContext(nc) as tc:
        emit_k2(tc, pre, wo, fin, D, TLOC, EOUT)
    return split_multi_waits(nc)


# =====================================================================
# Host orchestration
# =====================================================================
_CACHE = {}


def _install_ntff_hook():
    """Provide antenv.axon_hooks (absent in this image) so trace=True can
    capture NTFF profiles through the axon tunnel."""
    import sys, types

    if "antenv.axon_hooks" in sys.modules:
        return
    try:
        from trn_agent_boot.trn_boot import _ntff_profile_via_ctypes

        hook = _ntff_profile_via_ctypes("/opt/axon/libaxon_pjrt.so")
    except Exception:
        hook = None
    mod = types.ModuleType("antenv.axon_hooks")
    mod.get_axon_ntff_profile_hook = lambda: hook
    mod.set_axon_ntff_profile_hook = lambda h: None
    sys.modules["antenv.axon_hooks"] = mod


SPLIT3 = False


def _get_programs():
    if "k1" not in _CACHE:
        _CACHE["k1"] = build_k1(split3=SPLIT3)
        _CACHE["k2"] = build_k2()
    return _CACHE["k1"], _CACHE["k2"]


def _bf16_split(a):
    """a (f32) -> (hi, lo) bf16 with hi + lo ~= a to ~16 mantissa bits."""
    hi = a.astype(BF16_NP)
    lo = (a - hi.astype(np.float32)).astype(BF16_NP)
    return hi, lo


def _run_with_retry(nc, in_maps, trace, attempts=3):
    import time as _time

    last = None
    for i in range(attempts):
        try:
            return bass_utils.run_bass_kernel_spmd(
                nc, in_maps, core_ids=list(range(N_CORES)), trace=trace
            )
        except Exception as e:  # transient NRT / device errors
            last = e
            _time.sleep(2.0 * (i + 1))
    raise last


def kernel(x, Wq, Wk, Wv, Wo, _trace=False):
    x = np.asarray(x, dtype=np.float32)
    Wq = np.asarray(Wq, dtype=np.float32)
    Wk = np.asarray(Wk, dtype=np.float32)
    Wv = np.asarray(Wv, dtype=np.float32)
    Wo = np.asarray(Wo, dtype=np.float32)

    if _trace:
        _install_ntff_hook()

    nc1, nc2 = _get_programs()

    xt = np.ascontiguousarray(x.reshape(B * T, D_MODEL).T)  # [D, NTOK]
    ident = np.eye(128, dtype=BF16_NP)
    mask_t = np.triu(np.ones((128, 128), dtype=np.float32))  # [j,i]=1 if j<=i

    in_maps1 = []
    if not SPLIT3:
        xb = xt.astype(BF16_NP)
    if SPLIT3:
        xh, xl = _bf16_split(xt)
        for c in range(N_CORES):
            hs = slice(c * H_LOC * 128, (c + 1) * H_LOC * 128)
            wqh, wql = _bf16_split(np.ascontiguousarray((Wq[hs, :] * SCALE).T))
            wkh, wkl = _bf16_split(np.ascontiguousarray(Wk[hs, :].T))
            wvh, wvl = _bf16_split(np.ascontiguousarray(Wv[hs, :].T))
            in_maps1.append(
                {
                    "xh": xh, "xl": xl,
                    "wqh": wqh, "wql": wql,
                    "wkh": wkh, "wkl": wkl,
                    "wvh": wvh, "wvl": wvl,
                    "ident": ident, "mask": mask_t,
                }
            )
    else:
        for c in range(N_CORES):
            hs = slice(c * H_LOC * 128, (c + 1) * H_LOC * 128)
            in_maps1.append(
                {
                    "xt": xt,
                    "xb": xb,
                    "wq": np.ascontiguousarray((Wq[hs, :] * SCALE).T).astype(
                        BF16_NP
                    ),
                    "wkv": np.ascontiguousarray(
                        np.concatenate([Wk[hs, :].T, Wv[hs, :].T], axis=1)
                    ),
                    "ident": ident,
                    "mask": mask_t,
                }
            )
    res1 = _run_with_retry(nc1, in_maps1, _trace)
    t1 = res1.exec_time_ns

    # reshard: stack per-core [HL*128, NTOK] -> [D, NTOK], slice tokens
    pre_full = np.concatenate(
        [np.asarray(res1.results[c]["outp"]) for c in range(N_CORES)], axis=0
    )
    wo_t = np.ascontiguousarray(Wo.T).astype(BF16_NP)
    TLOC = B * T // N_CORES
    in_maps2 = []
    for c in range(N_CORES):
        in_maps2.append(
            {
                "pre": np.ascontiguousarray(pre_full[:, c * TLOC : (c + 1) * TLOC]),
                "wo": wo_t,
            }
        )
    res2 = _run_with_retry(nc2, in_maps2, _trace)
    t2 = res2.exec_time_ns

    out = np.concatenate(
        [np.asarray(res2.results[c]["fin"]) for c in range(N_CORES)], axis=0
    ).reshape(B, T, D_MODEL)

    fst = np.empty((B, N_HEADS, D_HEAD, D_HEAD), dtype=np.float32)
    for c in range(N_CORES):
        fst[:, c * H_LOC : (c + 1) * H_LOC] = res1.results[c]["fstate"]

    if _trace:
        kernel.last_exec_ns = ((t1 or 0), (t2 or 0))
    return out, fst
